# revision 32
# baseline (speedup 1.0000x reference)
"""Trainium2 Bass kernel for nn_Adapter_30674656428557 (GNN message passing).

Strategy (8 NeuronCores, SPMD, no collectives):
  - Nodes sharded by range: core c owns nodes [c*6250, (c+1)*6250).
  - Edges sharded by source node (the only node index the model uses), so
    every core computes its nodes' scatter-mean fully locally.
  - Host-side prep turns the irregular scatter into dense matmuls: within
    each 1024-node chunk, nodes are sorted by degree (desc) and edges laid
    out "level-major": level j holds the j-th edge-PAIR of every node that
    has one.  Level capacities use a data-INDEPENDENT envelope (Poisson
    order statistics + margin) so the compiled graph is identical across
    cores and runs (asserted against the real data at runtime).
  - Device, per chunk:
      time_feat = relu(edge_attr @ W_time.T): bf16 matmuls, even/odd column
                  streams on separate PE row-groups (concurrent), PSUM f32
                  evacuated with relu to bf16 SBUF in 1024-col strips
                  alternating between VectorE and ScalarE.
      scatter   = PSUM-accumulated matmuls over levels with a constant
                  stacked-identity stationary [I64; I64] that also folds
                  the two edges of each pair; even-node and odd-node
                  accumulators are column-tiled into one PSUM bank so the
                  two matmul streams run concurrently.
      mean      = one tensor_mul per chunk against host-provided
                  1/max(deg,1) stacked [even;odd] across partitions.
    Node MLPs are bf16 matmuls emitted per-chunk so the PE stream stays
    dense (HAM stays un-throttled); the residual is an identity matmul of
    x accumulated into the up-proj PSUM (bf16-rounded x: rel err ~2e-3,
    well under the 2e-2 gate).
"""

import math
import sys
from contextlib import ExitStack

import numpy as np

sys.path.insert(0, "/opt/trn_rl_repo")

from concourse import bacc, mybir, tile  # noqa: E402
from concourse.bass_utils import run_bass_kernel_spmd  # noqa: E402

DT = mybir.dt
BF = DT.bfloat16
F32 = DT.float32
NPBF = DT.np(BF)

N_NODES = 50000
N_EDGES = 1600000
IN_CH = 256
ADAPTER = 64
EDGE_DIM = 32

NCORES = 8
NC_NODES = N_NODES // NCORES     # 6250
CHUNK = 1024
NFULL = NC_NODES // CHUNK        # 6 full chunks
TAIL = NC_NODES - NFULL * CHUNK  # 106
NCH = NFULL + 1
BLK = 512
NBLOCKS = 2 * NCH                # 14 storage blocks of 512
N_STORE = NBLOCKS * BLK          # 7168
N_NTF = NCH * BLK                # 3584 (chunk-stacked [even;odd] layout)
LAM = N_EDGES / N_NODES          # 32.0

MAXLEV = 48


def _poisson_sf_odd(maxlev: int) -> np.ndarray:
    """P(deg >= 2j+1) for j = 0..maxlev-1, deg ~ Poisson(LAM)."""
    K = 400
    pmf = np.zeros(K, dtype=np.float64)
    pmf[0] = math.exp(-LAM)
    for k in range(1, K):
        pmf[k] = pmf[k - 1] * LAM / k
    sf = pmf[::-1].cumsum()[::-1]
    return np.array([sf[2 * j + 1] for j in range(maxlev)])


def _envelope(n_nodes: int) -> list:
    p = _poisson_sf_odd(MAXLEV)
    mean = n_nodes * p
    sig = np.sqrt(np.maximum(n_nodes * p * (1.0 - p), 0.0))
    env = mean + 4.0 * sig + 8.0
    caps = []
    for j in range(MAXLEV):
        c = int(math.ceil(env[j] / 16.0)) * 16
        c = max(c, 16)
        c = min(c, n_nodes)
        caps.append(c)
    caps[0] = n_nodes
    for j in range(1, MAXLEV):
        caps[j] = min(caps[j], caps[j - 1])
    keep = MAXLEV
    while keep > 1 and mean[keep - 1] < 1e-4:
        keep -= 1
    keep = min(MAXLEV, keep + 2)
    return caps[:keep]


CAPS_FULL = _envelope(CHUNK)
CAPS_TAIL = _envelope(TAIL)
LCOLS_FULL = [(c + 1) // 2 for c in CAPS_FULL]
LCOLS_TAIL = [(c + 1) // 2 for c in CAPS_TAIL]
C_FULL = sum(LCOLS_FULL)
C_TAIL = sum(LCOLS_TAIL)
C_TOT = NFULL * C_FULL + C_TAIL

CHBASE = [ch * C_FULL for ch in range(NFULL)] + [NFULL * C_FULL]
LBASE_FULL = np.concatenate([[0], np.cumsum(LCOLS_FULL)[:-1]]).astype(np.int64)
LBASE_TAIL = np.concatenate([[0], np.cumsum(LCOLS_TAIL)[:-1]]).astype(np.int64)

# attr DMA strip blocking: strips of 1024 cols, padded per chunk so each
# strip is a contiguous [128, 1024] DRAM block
STRIP = 1024
NSTRIP_FULL = (C_FULL + STRIP - 1) // STRIP
NSTRIP_TAIL = (C_TAIL + STRIP - 1) // STRIP
NSTRIPS = NFULL * NSTRIP_FULL + NSTRIP_TAIL

AVAIL_FULL = np.array([sum(1 for c in CAPS_FULL if c > s) for s in range(CHUNK)])
AVAIL_TAIL = np.array([sum(1 for c in CAPS_TAIL if c > s) for s in range(TAIL)])

_GRAPH_CACHE = {}


def _build_graph():
    if "nc" in _GRAPH_CACHE:
        return _GRAPH_CACHE["nc"]

    nc = bacc.Bacc("TRN2", target_bir_lowering=False, debug=False,
                   num_devices=NCORES)

    attr_d = nc.dram_tensor("attr2", [NSTRIPS * 128, STRIP], BF,
                            kind="ExternalInput").ap()
    xt0_d = nc.dram_tensor("xt0", [128, N_STORE], BF, kind="ExternalInput").ap()
    xt1_d = nc.dram_tensor("xt1", [128, N_STORE], BF, kind="ExternalInput").ap()
    rec_d = nc.dram_tensor("rec", [128, N_NTF], BF, kind="ExternalInput").ap()
    w2_d = nc.dram_tensor("w2", [128, 128], BF, kind="ExternalInput").ap()
    wd_d = nc.dram_tensor("wd", [128, 128], BF, kind="ExternalInput").ap()
    wf_d = nc.dram_tensor("wf", [128, 192], BF, kind="ExternalInput").ap()
    wu_d = nc.dram_tensor("wu", [64, 256], BF, kind="ExternalInput").ap()
    fold_d = nc.dram_tensor("fold", [128, 64], BF, kind="ExternalInput").ap()
    eye_d = nc.dram_tensor("eyeI", [128, 128], BF, kind="ExternalInput").ap()
    bias_d = nc.dram_tensor("biases", [128, 4], F32, kind="ExternalInput").ap()
    out_d = nc.dram_tensor("out", [256, N_STORE], F32, kind="ExternalOutput").ap()

    Relu = mybir.ActivationFunctionType.Relu
    Ident = mybir.ActivationFunctionType.Identity

    with tile.TileContext(nc) as tc, ExitStack() as ctx:
        consts = ctx.enter_context(tc.tile_pool(name="consts", bufs=1))
        attr_pool = ctx.enter_context(tc.tile_pool(name="attr", bufs=12))
        tf_pool = ctx.enter_context(tc.tile_pool(name="tf", bufs=3))
        small = ctx.enter_context(tc.tile_pool(name="small", bufs=2))
        outp = ctx.enter_context(tc.tile_pool(name="outp", bufs=4))
        ps_tf = ctx.enter_context(tc.tile_pool(name="ps_tf", bufs=5, space="PSUM"))
        ps_acc = ctx.enter_context(tc.tile_pool(name="ps_acc", bufs=2, space="PSUM"))
        ps_mlp = ctx.enter_context(tc.tile_pool(name="ps_mlp", bufs=1, space="PSUM"))

        w2 = consts.tile([128, 128], BF)
        nc.sync.dma_start(w2[:], w2_d[:])
        wd = consts.tile([128, 128], BF)
        nc.sync.dma_start(wd[:], wd_d[:])
        wf = consts.tile([128, 192], BF)
        nc.sync.dma_start(wf[:], wf_d[:])
        wu = consts.tile([64, 256], BF)
        nc.sync.dma_start(wu[:], wu_d[:])
        fold = consts.tile([128, 64], BF)
        nc.sync.dma_start(fold[:], fold_d[:])
        biases = consts.tile([128, 4], F32)
        nc.sync.dma_start(biases[:], bias_d[:])
        xt0 = consts.tile([128, N_STORE], BF)
        xt1 = consts.tile([128, N_STORE], BF)
        rec = consts.tile([128, N_NTF], BF)
        # cmb: even-parity blocks, [0:64]=node_feat, [64:128]=node_time_feat
        # nfo/nto: odd-parity blocks' node_feat / node_time_feat
        cmb = consts.tile([128, N_NTF], BF)
        nfo = consts.tile([64, N_NTF], BF)
        nto = consts.tile([64, N_NTF], BF)
        nc.gpsimd.memset(nto[:], 0.0)
        nc.gpsimd.memset(nfo[:], 0.0)
        nc.gpsimd.memset(cmb[:], 0.0)

        b_down = biases[0:64, 0:1]
        b_fus = biases[0:64, 1:2]

        flip = 0
        for ch in range(NCH):
            caps = CAPS_FULL if ch < NFULL else CAPS_TAIL
            lbase = LBASE_FULL if ch < NFULL else LBASE_TAIL
            Cc = C_FULL if ch < NFULL else C_TAIL
            cbase = CHBASE[ch]

            tfa = tf_pool.tile([128, C_FULL], BF, tag="tfa")
            tfb = tf_pool.tile([128, C_FULL], BF, tag="tfb")

            xsl = slice(2 * ch * BLK, (2 * ch + 2) * BLK)
            nc.sync.dma_start(xt0[:, xsl], xt0_d[:, xsl])
            nc.sync.dma_start(xt1[:, xsl], xt1_d[:, xsl])
            rsl = slice(ch * BLK, (ch + 1) * BLK)
            nc.sync.dma_start(rec[:, rsl], rec_d[:, rsl])

            # time_feat: DMA 1024-col strips, 2 matmuls per parity per strip
            # (even stream on PE rows 0-63, odd on rows 64-127, concurrent),
            # then one 1024-wide relu evacuation per parity.
            # per-level end columns so scatter matmuls can be emitted as
            # soon as the strips covering them are evacuated
            caps_l = list(caps)
            nlev = len(caps_l)
            lend = [int(lbase[j]) + (caps_l[j] + 1) // 2 for j in range(nlev)]
            next_lvl = 0
            acc_e = ps_acc.tile([128, BLK], F32, tag="acc")
            acc_o = ps_acc.tile([128, BLK], F32, tag="acc")

            strip0 = ch * NSTRIP_FULL  # tail chunk starts at NFULL*NSTRIP_FULL
            for si in range((Cc + STRIP - 1) // STRIP):
                s0 = si * STRIP
                w_ = min(STRIP, Cc - s0)
                r0 = (strip0 + si) * 128
                at = attr_pool.tile([128, STRIP], BF, tag="attr")
                nc.sync.dma_start(at[:, 0:w_], attr_d[r0:r0 + 128, 0:w_])
                for h0 in range(0, w_, 512):
                    h1 = min(h0 + 512, w_)
                    hw = h1 - h0
                    pe_ = ps_tf.tile([128, 512], F32, tag="ps_tf")
                    po_ = ps_tf.tile([128, 512], F32, tag="ps_tf")
                    nc.tensor.matmul(pe_[:, 0:hw], w2[0:64, :], at[0:64, h0:h1])
                    nc.tensor.matmul(po_[:, 0:hw], w2[64:128, :], at[64:128, h0:h1])
                    d0 = s0 + h0
                    if flip == 0:
                        nc.vector.tensor_scalar_max(tfa[:, d0:d0 + hw],
                                                    pe_[:, 0:hw], 0.0)
                        nc.scalar.activation(tfb[:, d0:d0 + hw], po_[:, 0:hw], Relu)
                    else:
                        nc.scalar.activation(tfa[:, d0:d0 + hw], pe_[:, 0:hw], Relu)
                        nc.vector.tensor_scalar_max(tfb[:, d0:d0 + hw],
                                                    po_[:, 0:hw], 0.0)
                    flip ^= 1

                # scatter levels fully covered by the strips evacuated so
                # far: even-node sums -> acc_e[64:128] (col-group 64),
                # odd-node sums -> acc_o[0:64] (col-group 0).  Separate
                # PSUM banks, different column groups -> the two matmul
                # streams run concurrently on the PE.  (A single
                # accumulation group must keep ONE tile_position: mixed
                # row/col groups within a group fault on hardware.)
                covered = min(s0 + STRIP, Cc)
                while next_lvl < nlev and lend[next_lvl] <= covered:
                    j = next_lvl
                    nA = (caps_l[j] + 1) // 2
                    nB = caps_l[j] // 2
                    c0 = int(lbase[j])
                    nc.tensor.matmul(acc_e[64:128, 0:nA], fold[:],
                                     tfa[:, c0:c0 + nA],
                                     start=(j == 0), stop=(j == nlev - 1))
                    nc.tensor.matmul(acc_o[0:64, 0:nB], fold[:],
                                     tfb[:, c0:c0 + nB],
                                     start=(j == 0), stop=(j == nlev - 1))
                    next_lvl += 1
            assert next_lvl == nlev, (next_lvl, nlev)
            nA0 = (caps[0] + 1) // 2
            nB0 = caps[0] // 2

            # mean: even-parity -> cmb[64:128] (lanes 64-127), odd -> nto
            nt0 = ch * BLK
            nc.vector.tensor_mul(cmb[64:128, nt0:nt0 + nA0], acc_e[64:128, 0:nA0],
                                 rec[64:128, nt0:nt0 + nA0])
            nc.vector.tensor_mul(nto[0:64, nt0:nt0 + nB0], acc_o[0:64, 0:nB0],
                                 rec[0:64, nt0:nt0 + nB0])

            # node MLP for this chunk's two storage blocks (even b, odd b)
            for par in range(2):
                b = 2 * ch + par
                sl = slice(b * BLK, (b + 1) * BLK)
                psn = ps_mlp.tile([128, BLK], F32, tag="mlp")
                nc.tensor.matmul(psn[0:64, :], wd[:, 0:64], xt0[:, sl],
                                 start=True, stop=False)
                nc.tensor.matmul(psn[0:64, :], wd[:, 64:128], xt1[:, sl],
                                 start=False, stop=True)
                nf_dst = cmb if par == 0 else nfo
                nc.scalar.activation(nf_dst[0:64, nt0:nt0 + BLK], psn[0:64, :],
                                     Relu, bias=b_down)

                psf = ps_mlp.tile([128, BLK], F32, tag="mlp")
                if par == 0:
                    # single K=128 matmul over stacked [nf; ntf]
                    nc.tensor.matmul(psf[0:64, :], wf[:, 0:64],
                                     cmb[:, nt0:nt0 + BLK])
                else:
                    # two K=64 matmuls, both on PE rows 0-63
                    nc.tensor.matmul(psf[0:64, :], wf[0:64, 64:128],
                                     nfo[:, nt0:nt0 + BLK],
                                     start=True, stop=False)
                    nc.tensor.matmul(psf[0:64, :], wf[0:64, 128:192],
                                     nto[:, nt0:nt0 + BLK],
                                     start=False, stop=True)
                fused = small.tile([64, BLK], BF, tag="fused")
                nc.scalar.activation(fused[:], psf[0:64, :], Relu, bias=b_fus)
                for h in range(2):
                    psu = ps_mlp.tile([128, BLK], F32, tag="mlp")
                    nc.tensor.matmul(psu[:], wu[:, 128 * h:128 * (h + 1)], fused[:])
                    xth = xt0 if h == 0 else xt1
                    ob = outp.tile([128, BLK], F32, tag="ob")
                    # residual + bias fused into the evacuation:
                    # ob = (psu + b_up_h) + x
                    nc.vector.scalar_tensor_tensor(
                        ob[:], psu[:], biases[:, 2 + h:3 + h], xth[:, sl],
                        op0=mybir.AluOpType.add, op1=mybir.AluOpType.add)
                    nc.sync.dma_start(out_d[128 * h:128 * (h + 1), sl], ob[:])

    nc.compile()
    _GRAPH_CACHE["nc"] = nc
    return nc


def prepare(x, edge_index, edge_attr, W_down, b_down, W_time, b_time,
            W_fusion, b_fusion, W_up, b_up):
    """Host-side sharding/layout. Returns (in_maps, store_cols[NCORES, NC_NODES])."""
    x = np.asarray(x, dtype=np.float32)
    edge_index = np.asarray(edge_index)
    edge_attr = np.asarray(edge_attr, dtype=np.float32)
    W_down = np.asarray(W_down, dtype=np.float32)
    b_down = np.asarray(b_down, dtype=np.float32)
    W_time = np.asarray(W_time, dtype=np.float32)
    b_time = np.asarray(b_time, dtype=np.float32)
    W_fusion = np.asarray(W_fusion, dtype=np.float32)
    b_fusion = np.asarray(b_fusion, dtype=np.float32)
    W_up = np.asarray(W_up, dtype=np.float32)
    b_up = np.asarray(b_up, dtype=np.float32)

    assert not np.any(b_time), "ghost slots in the padded layout assume b_time == 0"

    src = edge_index[0].astype(np.int64)
    deg = np.bincount(src, minlength=N_NODES).astype(np.int64)

    # per-node: within-chunk degree-sorted position and storage column
    s_pos = np.empty(N_NODES, dtype=np.int64)
    for c in range(NCORES):
        for ch in range(NCH):
            lo = c * NC_NODES + ch * CHUNK
            hi = min(c * NC_NODES + (ch + 1) * CHUNK, (c + 1) * NC_NODES)
            order = np.argsort(-deg[lo:hi], kind="stable")
            s = np.empty(hi - lo, dtype=np.int64)
            s[order] = np.arange(hi - lo)
            s_pos[lo:hi] = s
    ln = np.arange(N_NODES) % NC_NODES
    chn = ln // CHUNK
    store_col = chn * (2 * BLK) + (s_pos % 2) * BLK + s_pos // 2
    # ntf/rec layout: chunk-stacked, partition half = s_pos parity
    ntf_col = chn * BLK + s_pos // 2

    # envelope fit check
    w_pairs = (deg + 1) // 2
    is_tail = chn == NFULL
    avail = np.where(is_tail, AVAIL_TAIL[np.minimum(s_pos, TAIL - 1)],
                     AVAIL_FULL[np.minimum(s_pos, CHUNK - 1)])
    if np.any(w_pairs > avail):
        raise RuntimeError(
            f"envelope too tight: {int(np.sum(w_pairs > avail))} nodes exceed capacity")

    # per-edge placement
    esort = np.argsort(src, kind="stable")
    starts = np.zeros(N_NODES + 1, dtype=np.int64)
    np.cumsum(deg, out=starts[1:])
    srcs = src[esort]
    rank = np.arange(N_EDGES, dtype=np.int64) - starts[srcs]
    q = rank // 2
    par = rank % 2
    chv = chn[srcs]
    lb = np.where(chv == NFULL,
                  LBASE_TAIL[np.minimum(q, len(LCOLS_TAIL) - 1)],
                  LBASE_FULL[np.minimum(q, len(LCOLS_FULL) - 1)])
    colc = np.array(CHBASE, dtype=np.int64)[chv] + lb + s_pos[srcs] // 2
    halfq = s_pos[srcs] % 2
    slot = colc * 4 + halfq * 2 + par
    core_of_edge = srcs // NC_NODES

    ea_bf = edge_attr.astype(NPBF)

    # shared weights
    bd = np.zeros((64, 128), dtype=np.float32)
    bd[0:32, 0:64] = W_time.T
    bd[32:64, 64:128] = W_time.T
    w2 = np.concatenate([bd, bd], axis=0).astype(NPBF)                 # [128,128]
    wd = np.concatenate([W_down.T[0:128], W_down.T[128:256]], axis=1).astype(NPBF)
    wfT = W_fusion.T                                                    # [128, 64]
    wf = np.zeros((128, 192), dtype=np.float32)
    wf[:, 0:64] = wfT                 # stacked [nf; ntf] for even blocks
    wf[0:64, 64:128] = wfT[0:64]      # nf part for odd blocks (rows 0-63)
    wf[0:64, 128:192] = wfT[64:128]   # ntf part for odd blocks (rows 0-63)
    wf = wf.astype(NPBF)
    wu = W_up.T.astype(NPBF)                                            # [64, 256]
    fold = np.concatenate([np.eye(64), np.eye(64)], axis=0).astype(NPBF)
    eyeI = np.eye(128, dtype=np.float32).astype(NPBF)
    biases = np.zeros((128, 4), dtype=np.float32)
    biases[0:64, 0] = b_down
    biases[0:64, 1] = b_fusion
    biases[:, 2] = b_up[0:128]
    biases[:, 3] = b_up[128:256]

    in_maps = []
    store_cols = np.empty((NCORES, NC_NODES), dtype=np.int64)
    for c in range(NCORES):
        em = core_of_edge == c
        attr_flat = np.zeros((C_TOT * 4, EDGE_DIM), dtype=NPBF)
        attr_flat[slot[em]] = ea_bf[esort[em]]
        attr2 = np.ascontiguousarray(
            attr_flat.reshape(C_TOT, 4, EDGE_DIM)
            .transpose(1, 2, 0).reshape(128, C_TOT))
        # strip-blocked layout: each [128, 1024] strip contiguous in DRAM
        attr_blk = np.zeros((NSTRIPS, 128, STRIP), dtype=NPBF)
        for ch in range(NCH):
            Cc = C_FULL if ch < NFULL else C_TAIL
            ns = NSTRIP_FULL if ch < NFULL else NSTRIP_TAIL
            cb = CHBASE[ch]
            s0 = ch * NSTRIP_FULL
            blk = np.zeros((128, ns * STRIP), dtype=NPBF)
            blk[:, 0:Cc] = attr2[:, cb:cb + Cc]
            attr_blk[s0:s0 + ns] = blk.reshape(128, ns, STRIP).transpose(1, 0, 2)
        attr_blk = attr_blk.reshape(NSTRIPS * 128, STRIP)

        nlo = c * NC_NODES
        st = store_col[nlo:nlo + NC_NODES]
        store_cols[c] = st
        xst = np.zeros((N_STORE, IN_CH), dtype=np.float32)
        xst[st] = x[nlo:nlo + NC_NODES]
        xt = np.ascontiguousarray(xst.T).astype(NPBF)

        recv = 1.0 / np.maximum(deg[nlo:nlo + NC_NODES], 1)
        rec128 = np.zeros((2, 64, N_NTF), dtype=np.float32)
        nti = ntf_col[nlo:nlo + NC_NODES]
        pari = s_pos[nlo:nlo + NC_NODES] % 2
        # even-parity nodes live on partitions 64-127, odd on 0-63
        rec128[1 - pari, :, nti] = recv[:, None]
        rec128 = np.ascontiguousarray(rec128.reshape(128, N_NTF)).astype(NPBF)

        in_maps.append({
            "attr2": attr_blk,
            "xt0": np.ascontiguousarray(xt[0:128]),
            "xt1": np.ascontiguousarray(xt[128:256]),
            "rec": rec128,
            "w2": w2,
            "wd": wd,
            "wf": wf,
            "wu": wu,
            "fold": fold,
            "eyeI": eyeI,
            "biases": biases,
        })
    return in_maps, store_cols


def run(in_maps, trace=False, **kw):
    nc = _build_graph()
    return run_bass_kernel_spmd(nc, in_maps, core_ids=list(range(NCORES)),
                                trace=trace, **kw)


def unshard(results, store_cols):
    out = np.empty((N_NODES, IN_CH), dtype=np.float32)
    for c in range(NCORES):
        o = np.asarray(results[c]["out"], dtype=np.float32)  # [256, N_STORE]
        out[c * NC_NODES:(c + 1) * NC_NODES] = o[:, store_cols[c]].T
    return out


def kernel(**inputs):
    in_maps, store_cols = prepare(**inputs)
    res = run(in_maps, trace=False)
    return unshard(res.results, store_cols)


# revision 33
# speedup vs baseline: 1.1813x; 1.1813x over previous
"""Trainium2 Bass kernel for nn_Adapter_30674656428557 (GNN message passing).

Strategy (8 NeuronCores, SPMD, no collectives):
  - Nodes sharded by range: core c owns nodes [c*6250, (c+1)*6250).
  - Edges sharded by source node (the only node index the model uses), so
    every core computes its nodes' scatter-mean fully locally.
  - Host-side prep turns the irregular scatter into dense matmuls: within
    each 1024-node chunk, nodes are sorted by degree (desc) and edges laid
    out "level-major": level j holds the j-th edge-PAIR of every node that
    has one.  Level capacities use a data-INDEPENDENT envelope (Poisson
    order statistics + margin) so the compiled graph is identical across
    cores and runs (asserted against the real data at runtime).
  - Device, per chunk:
      time_feat = relu(edge_attr @ W_time.T): bf16 matmuls, even/odd column
                  streams on separate PE row-groups (concurrent), PSUM f32
                  evacuated with relu to bf16 SBUF in 1024-col strips
                  alternating between VectorE and ScalarE.
      scatter   = PSUM-accumulated matmuls over levels with a constant
                  stacked-identity stationary [I64; I64] that also folds
                  the two edges of each pair; even-node and odd-node
                  accumulators are column-tiled into one PSUM bank so the
                  two matmul streams run concurrently.
      mean      = one tensor_mul per chunk against host-provided
                  1/max(deg,1) stacked [even;odd] across partitions.
    Node MLPs are bf16 matmuls emitted per-chunk so the PE stream stays
    dense (HAM stays un-throttled); the residual is an identity matmul of
    x accumulated into the up-proj PSUM (bf16-rounded x: rel err ~2e-3,
    well under the 2e-2 gate).
"""

import math
import sys
from contextlib import ExitStack

import numpy as np

sys.path.insert(0, "/opt/trn_rl_repo")

from concourse import bacc, mybir, tile  # noqa: E402
from concourse.bass_utils import run_bass_kernel_spmd  # noqa: E402

DT = mybir.dt
BF = DT.bfloat16
F32 = DT.float32
NPBF = DT.np(BF)

N_NODES = 50000
N_EDGES = 1600000
IN_CH = 256
ADAPTER = 64
EDGE_DIM = 32

NCORES = 8
NC_NODES = N_NODES // NCORES     # 6250
CHUNK = 1024
NFULL = NC_NODES // CHUNK        # 6 full chunks
TAIL = NC_NODES - NFULL * CHUNK  # 106
NCH = NFULL + 1
BLK = 512
NBLOCKS = 2 * NCH                # 14 storage blocks of 512
N_STORE = NBLOCKS * BLK          # 7168
N_NTF = NCH * BLK                # 3584 (chunk-stacked [even;odd] layout)
LAM = N_EDGES / N_NODES          # 32.0

MAXLEV = 48


def _poisson_sf_odd(maxlev: int) -> np.ndarray:
    """P(deg >= 2j+1) for j = 0..maxlev-1, deg ~ Poisson(LAM)."""
    K = 400
    pmf = np.zeros(K, dtype=np.float64)
    pmf[0] = math.exp(-LAM)
    for k in range(1, K):
        pmf[k] = pmf[k - 1] * LAM / k
    sf = pmf[::-1].cumsum()[::-1]
    return np.array([sf[2 * j + 1] for j in range(maxlev)])


def _envelope(n_nodes: int) -> list:
    p = _poisson_sf_odd(MAXLEV)
    mean = n_nodes * p
    sig = np.sqrt(np.maximum(n_nodes * p * (1.0 - p), 0.0))
    env = mean + 4.0 * sig + 8.0
    caps = []
    for j in range(MAXLEV):
        c = int(math.ceil(env[j] / 16.0)) * 16
        c = max(c, 16)
        c = min(c, n_nodes)
        caps.append(c)
    caps[0] = n_nodes
    for j in range(1, MAXLEV):
        caps[j] = min(caps[j], caps[j - 1])
    keep = MAXLEV
    while keep > 1 and mean[keep - 1] < 1e-4:
        keep -= 1
    keep = min(MAXLEV, keep + 2)
    return caps[:keep]


CAPS_FULL = _envelope(CHUNK)
CAPS_TAIL = _envelope(TAIL)
LCOLS_FULL = [(c + 1) // 2 for c in CAPS_FULL]
LCOLS_TAIL = [(c + 1) // 2 for c in CAPS_TAIL]
C_FULL = sum(LCOLS_FULL)
C_TAIL = sum(LCOLS_TAIL)
C_TOT = NFULL * C_FULL + C_TAIL

CHBASE = [ch * C_FULL for ch in range(NFULL)] + [NFULL * C_FULL]
LBASE_FULL = np.concatenate([[0], np.cumsum(LCOLS_FULL)[:-1]]).astype(np.int64)
LBASE_TAIL = np.concatenate([[0], np.cumsum(LCOLS_TAIL)[:-1]]).astype(np.int64)

# attr DMA strip blocking: strips of 1024 cols, padded per chunk so each
# strip is a contiguous [128, 1024] DRAM block
STRIP = 1024
NSTRIP_FULL = (C_FULL + STRIP - 1) // STRIP
NSTRIP_TAIL = (C_TAIL + STRIP - 1) // STRIP
NSTRIPS = NFULL * NSTRIP_FULL + NSTRIP_TAIL

AVAIL_FULL = np.array([sum(1 for c in CAPS_FULL if c > s) for s in range(CHUNK)])
AVAIL_TAIL = np.array([sum(1 for c in CAPS_TAIL if c > s) for s in range(TAIL)])

_GRAPH_CACHE = {}


def _build_graph():
    if "nc" in _GRAPH_CACHE:
        return _GRAPH_CACHE["nc"]

    nc = bacc.Bacc("TRN2", target_bir_lowering=False, debug=False,
                   num_devices=NCORES)

    attr_d = nc.dram_tensor("attr2", [NSTRIPS * 128, STRIP], BF,
                            kind="ExternalInput").ap()
    xt0_d = nc.dram_tensor("xt0", [128, N_STORE], BF, kind="ExternalInput").ap()
    xt1_d = nc.dram_tensor("xt1", [128, N_STORE], BF, kind="ExternalInput").ap()
    rec_d = nc.dram_tensor("rec", [128, N_NTF], BF, kind="ExternalInput").ap()
    w2_d = nc.dram_tensor("w2", [128, 128], BF, kind="ExternalInput").ap()
    wd_d = nc.dram_tensor("wd", [128, 128], BF, kind="ExternalInput").ap()
    wf_d = nc.dram_tensor("wf", [128, 192], BF, kind="ExternalInput").ap()
    wu_d = nc.dram_tensor("wu", [64, 256], BF, kind="ExternalInput").ap()
    fold_d = nc.dram_tensor("fold", [128, 64], BF, kind="ExternalInput").ap()
    eye_d = nc.dram_tensor("eyeI", [128, 128], BF, kind="ExternalInput").ap()
    bias_d = nc.dram_tensor("biases", [128, 4], F32, kind="ExternalInput").ap()
    out_d = nc.dram_tensor("out", [256, N_STORE], F32, kind="ExternalOutput").ap()

    Relu = mybir.ActivationFunctionType.Relu
    Ident = mybir.ActivationFunctionType.Identity

    with tile.TileContext(nc) as tc, ExitStack() as ctx:
        consts = ctx.enter_context(tc.tile_pool(name="consts", bufs=1))
        attr_pool = ctx.enter_context(tc.tile_pool(name="attr", bufs=12))
        tf_pool = ctx.enter_context(tc.tile_pool(name="tf", bufs=3))
        small = ctx.enter_context(tc.tile_pool(name="small", bufs=2))
        outp = ctx.enter_context(tc.tile_pool(name="outp", bufs=4))
        ps_tf = ctx.enter_context(tc.tile_pool(name="ps_tf", bufs=5, space="PSUM"))
        ps_acc = ctx.enter_context(tc.tile_pool(name="ps_acc", bufs=2, space="PSUM"))
        ps_mlp = ctx.enter_context(tc.tile_pool(name="ps_mlp", bufs=1, space="PSUM"))

        w2 = consts.tile([128, 128], BF)
        nc.sync.dma_start(w2[:], w2_d[:])
        wd = consts.tile([128, 128], BF)
        nc.sync.dma_start(wd[:], wd_d[:])
        wf = consts.tile([128, 192], BF)
        nc.sync.dma_start(wf[:], wf_d[:])
        wu = consts.tile([64, 256], BF)
        nc.sync.dma_start(wu[:], wu_d[:])
        fold = consts.tile([128, 64], BF)
        nc.sync.dma_start(fold[:], fold_d[:])
        biases = consts.tile([128, 4], F32)
        nc.sync.dma_start(biases[:], bias_d[:])
        xt0 = consts.tile([128, N_STORE], BF)
        xt1 = consts.tile([128, N_STORE], BF)
        rec = consts.tile([128, N_NTF], BF)
        # cmb: even-parity blocks, [0:64]=node_feat, [64:128]=node_time_feat
        # nfo/nto: odd-parity blocks' node_feat / node_time_feat
        cmb = consts.tile([128, N_NTF], BF)
        nfo = consts.tile([64, N_NTF], BF)
        nto = consts.tile([64, N_NTF], BF)
        nc.gpsimd.memset(nto[:], 0.0)
        nc.gpsimd.memset(nfo[:], 0.0)
        nc.gpsimd.memset(cmb[:], 0.0)

        b_down = biases[0:64, 0:1]
        b_fus = biases[0:64, 1:2]

        flip = 0
        for ch in range(NCH):
            caps = CAPS_FULL if ch < NFULL else CAPS_TAIL
            lbase = LBASE_FULL if ch < NFULL else LBASE_TAIL
            Cc = C_FULL if ch < NFULL else C_TAIL
            cbase = CHBASE[ch]

            tfa = tf_pool.tile([128, C_FULL], BF, tag="tfa")
            tfb = tf_pool.tile([128, C_FULL], BF, tag="tfb")

            xsl = slice(2 * ch * BLK, (2 * ch + 2) * BLK)
            nc.sync.dma_start(xt0[:, xsl], xt0_d[:, xsl])
            nc.sync.dma_start(xt1[:, xsl], xt1_d[:, xsl])
            rsl = slice(ch * BLK, (ch + 1) * BLK)
            nc.sync.dma_start(rec[:, rsl], rec_d[:, rsl])

            # time_feat: DMA 1024-col strips, 2 matmuls per parity per strip
            # (even stream on PE rows 0-63, odd on rows 64-127, concurrent),
            # then one 1024-wide relu evacuation per parity.
            strip0 = ch * NSTRIP_FULL  # tail chunk starts at NFULL*NSTRIP_FULL
            for si in range((Cc + STRIP - 1) // STRIP):
                s0 = si * STRIP
                w_ = min(STRIP, Cc - s0)
                r0 = (strip0 + si) * 128
                at = attr_pool.tile([128, STRIP], BF, tag="attr")
                nc.sync.dma_start(at[:, 0:w_], attr_d[r0:r0 + 128, 0:w_])
                for h0 in range(0, w_, 512):
                    h1 = min(h0 + 512, w_)
                    hw = h1 - h0
                    pe_ = ps_tf.tile([128, 512], F32, tag="ps_tf")
                    po_ = ps_tf.tile([128, 512], F32, tag="ps_tf")
                    nc.tensor.matmul(pe_[:, 0:hw], w2[0:64, :], at[0:64, h0:h1])
                    nc.tensor.matmul(po_[:, 0:hw], w2[64:128, :], at[64:128, h0:h1])
                    d0 = s0 + h0
                    if flip == 0:
                        nc.vector.tensor_scalar_max(tfa[:, d0:d0 + hw],
                                                    pe_[:, 0:hw], 0.0)
                        nc.scalar.activation(tfb[:, d0:d0 + hw], po_[:, 0:hw], Relu)
                    else:
                        nc.scalar.activation(tfa[:, d0:d0 + hw], pe_[:, 0:hw], Relu)
                        nc.vector.tensor_scalar_max(tfb[:, d0:d0 + hw],
                                                    po_[:, 0:hw], 0.0)
                    flip ^= 1

            # scatter: even-node sums -> acc_e[64:128] (col-group 64),
            # odd-node sums -> acc_o[0:64] (col-group 0).  Separate PSUM
            # banks, different column groups -> the two matmul streams run
            # concurrently on the PE.  (A single accumulation group must
            # keep ONE tile_position: mixed row/col groups within a group
            # fault on hardware.)
            nA0 = (caps[0] + 1) // 2
            nB0 = caps[0] // 2
            acc_e = ps_acc.tile([128, BLK], F32, tag="acc")
            acc_o = ps_acc.tile([128, BLK], F32, tag="acc")
            nlev = len(caps)
            for j in range(nlev):
                nA = (caps[j] + 1) // 2
                nB = caps[j] // 2
                c0 = int(lbase[j])
                nc.tensor.matmul(acc_e[64:128, 0:nA], fold[:], tfa[:, c0:c0 + nA],
                                 start=(j == 0), stop=(j == nlev - 1))
                nc.tensor.matmul(acc_o[0:64, 0:nB], fold[:], tfb[:, c0:c0 + nB],
                                 start=(j == 0), stop=(j == nlev - 1))

            # mean: even-parity -> cmb[64:128] (lanes 64-127), odd -> nto
            nt0 = ch * BLK
            nc.vector.tensor_mul(cmb[64:128, nt0:nt0 + nA0], acc_e[64:128, 0:nA0],
                                 rec[64:128, nt0:nt0 + nA0])
            nc.vector.tensor_mul(nto[0:64, nt0:nt0 + nB0], acc_o[0:64, 0:nB0],
                                 rec[0:64, nt0:nt0 + nB0])

            # node MLP for this chunk's two storage blocks (even b, odd b)
            for par in range(2):
                b = 2 * ch + par
                sl = slice(b * BLK, (b + 1) * BLK)
                psn = ps_mlp.tile([128, BLK], F32, tag="mlp")
                nc.tensor.matmul(psn[0:64, :], wd[:, 0:64], xt0[:, sl],
                                 start=True, stop=False)
                nc.tensor.matmul(psn[0:64, :], wd[:, 64:128], xt1[:, sl],
                                 start=False, stop=True)
                nf_dst = cmb if par == 0 else nfo
                nc.scalar.activation(nf_dst[0:64, nt0:nt0 + BLK], psn[0:64, :],
                                     Relu, bias=b_down)

                psf = ps_mlp.tile([128, BLK], F32, tag="mlp")
                if par == 0:
                    # single K=128 matmul over stacked [nf; ntf]
                    nc.tensor.matmul(psf[0:64, :], wf[:, 0:64],
                                     cmb[:, nt0:nt0 + BLK])
                else:
                    # two K=64 matmuls, both on PE rows 0-63
                    nc.tensor.matmul(psf[0:64, :], wf[0:64, 64:128],
                                     nfo[:, nt0:nt0 + BLK],
                                     start=True, stop=False)
                    nc.tensor.matmul(psf[0:64, :], wf[0:64, 128:192],
                                     nto[:, nt0:nt0 + BLK],
                                     start=False, stop=True)
                fused = small.tile([64, BLK], BF, tag="fused")
                nc.scalar.activation(fused[:], psf[0:64, :], Relu, bias=b_fus)
                for h in range(2):
                    psu = ps_mlp.tile([128, BLK], F32, tag="mlp")
                    nc.tensor.matmul(psu[:], wu[:, 128 * h:128 * (h + 1)], fused[:])
                    xth = xt0 if h == 0 else xt1
                    ob = outp.tile([128, BLK], F32, tag="ob")
                    # residual + bias fused into the evacuation:
                    # ob = (psu + b_up_h) + x
                    nc.vector.scalar_tensor_tensor(
                        ob[:], psu[:], biases[:, 2 + h:3 + h], xth[:, sl],
                        op0=mybir.AluOpType.add, op1=mybir.AluOpType.add)
                    nc.sync.dma_start(out_d[128 * h:128 * (h + 1), sl], ob[:])

    nc.compile()
    _GRAPH_CACHE["nc"] = nc
    return nc


def prepare(x, edge_index, edge_attr, W_down, b_down, W_time, b_time,
            W_fusion, b_fusion, W_up, b_up):
    """Host-side sharding/layout. Returns (in_maps, store_cols[NCORES, NC_NODES])."""
    x = np.asarray(x, dtype=np.float32)
    edge_index = np.asarray(edge_index)
    edge_attr = np.asarray(edge_attr, dtype=np.float32)
    W_down = np.asarray(W_down, dtype=np.float32)
    b_down = np.asarray(b_down, dtype=np.float32)
    W_time = np.asarray(W_time, dtype=np.float32)
    b_time = np.asarray(b_time, dtype=np.float32)
    W_fusion = np.asarray(W_fusion, dtype=np.float32)
    b_fusion = np.asarray(b_fusion, dtype=np.float32)
    W_up = np.asarray(W_up, dtype=np.float32)
    b_up = np.asarray(b_up, dtype=np.float32)

    assert not np.any(b_time), "ghost slots in the padded layout assume b_time == 0"

    src = edge_index[0].astype(np.int64)
    deg = np.bincount(src, minlength=N_NODES).astype(np.int64)

    # per-node: within-chunk degree-sorted position and storage column
    s_pos = np.empty(N_NODES, dtype=np.int64)
    for c in range(NCORES):
        for ch in range(NCH):
            lo = c * NC_NODES + ch * CHUNK
            hi = min(c * NC_NODES + (ch + 1) * CHUNK, (c + 1) * NC_NODES)
            order = np.argsort(-deg[lo:hi], kind="stable")
            s = np.empty(hi - lo, dtype=np.int64)
            s[order] = np.arange(hi - lo)
            s_pos[lo:hi] = s
    ln = np.arange(N_NODES) % NC_NODES
    chn = ln // CHUNK
    store_col = chn * (2 * BLK) + (s_pos % 2) * BLK + s_pos // 2
    # ntf/rec layout: chunk-stacked, partition half = s_pos parity
    ntf_col = chn * BLK + s_pos // 2

    # envelope fit check
    w_pairs = (deg + 1) // 2
    is_tail = chn == NFULL
    avail = np.where(is_tail, AVAIL_TAIL[np.minimum(s_pos, TAIL - 1)],
                     AVAIL_FULL[np.minimum(s_pos, CHUNK - 1)])
    if np.any(w_pairs > avail):
        raise RuntimeError(
            f"envelope too tight: {int(np.sum(w_pairs > avail))} nodes exceed capacity")

    # per-edge placement
    esort = np.argsort(src, kind="stable")
    starts = np.zeros(N_NODES + 1, dtype=np.int64)
    np.cumsum(deg, out=starts[1:])
    srcs = src[esort]
    rank = np.arange(N_EDGES, dtype=np.int64) - starts[srcs]
    q = rank // 2
    par = rank % 2
    chv = chn[srcs]
    lb = np.where(chv == NFULL,
                  LBASE_TAIL[np.minimum(q, len(LCOLS_TAIL) - 1)],
                  LBASE_FULL[np.minimum(q, len(LCOLS_FULL) - 1)])
    colc = np.array(CHBASE, dtype=np.int64)[chv] + lb + s_pos[srcs] // 2
    halfq = s_pos[srcs] % 2
    slot = colc * 4 + halfq * 2 + par
    core_of_edge = srcs // NC_NODES

    ea_bf = edge_attr.astype(NPBF)

    # shared weights
    bd = np.zeros((64, 128), dtype=np.float32)
    bd[0:32, 0:64] = W_time.T
    bd[32:64, 64:128] = W_time.T
    w2 = np.concatenate([bd, bd], axis=0).astype(NPBF)                 # [128,128]
    wd = np.concatenate([W_down.T[0:128], W_down.T[128:256]], axis=1).astype(NPBF)
    wfT = W_fusion.T                                                    # [128, 64]
    wf = np.zeros((128, 192), dtype=np.float32)
    wf[:, 0:64] = wfT                 # stacked [nf; ntf] for even blocks
    wf[0:64, 64:128] = wfT[0:64]      # nf part for odd blocks (rows 0-63)
    wf[0:64, 128:192] = wfT[64:128]   # ntf part for odd blocks (rows 0-63)
    wf = wf.astype(NPBF)
    wu = W_up.T.astype(NPBF)                                            # [64, 256]
    fold = np.concatenate([np.eye(64), np.eye(64)], axis=0).astype(NPBF)
    eyeI = np.eye(128, dtype=np.float32).astype(NPBF)
    biases = np.zeros((128, 4), dtype=np.float32)
    biases[0:64, 0] = b_down
    biases[0:64, 1] = b_fusion
    biases[:, 2] = b_up[0:128]
    biases[:, 3] = b_up[128:256]

    in_maps = []
    store_cols = np.empty((NCORES, NC_NODES), dtype=np.int64)
    for c in range(NCORES):
        em = core_of_edge == c
        attr_flat = np.zeros((C_TOT * 4, EDGE_DIM), dtype=NPBF)
        attr_flat[slot[em]] = ea_bf[esort[em]]
        attr2 = np.ascontiguousarray(
            attr_flat.reshape(C_TOT, 4, EDGE_DIM)
            .transpose(1, 2, 0).reshape(128, C_TOT))
        # strip-blocked layout: each [128, 1024] strip contiguous in DRAM
        attr_blk = np.zeros((NSTRIPS, 128, STRIP), dtype=NPBF)
        for ch in range(NCH):
            Cc = C_FULL if ch < NFULL else C_TAIL
            ns = NSTRIP_FULL if ch < NFULL else NSTRIP_TAIL
            cb = CHBASE[ch]
            s0 = ch * NSTRIP_FULL
            blk = np.zeros((128, ns * STRIP), dtype=NPBF)
            blk[:, 0:Cc] = attr2[:, cb:cb + Cc]
            attr_blk[s0:s0 + ns] = blk.reshape(128, ns, STRIP).transpose(1, 0, 2)
        attr_blk = attr_blk.reshape(NSTRIPS * 128, STRIP)

        nlo = c * NC_NODES
        st = store_col[nlo:nlo + NC_NODES]
        store_cols[c] = st
        xst = np.zeros((N_STORE, IN_CH), dtype=np.float32)
        xst[st] = x[nlo:nlo + NC_NODES]
        xt = np.ascontiguousarray(xst.T).astype(NPBF)

        recv = 1.0 / np.maximum(deg[nlo:nlo + NC_NODES], 1)
        rec128 = np.zeros((2, 64, N_NTF), dtype=np.float32)
        nti = ntf_col[nlo:nlo + NC_NODES]
        pari = s_pos[nlo:nlo + NC_NODES] % 2
        # even-parity nodes live on partitions 64-127, odd on 0-63
        rec128[1 - pari, :, nti] = recv[:, None]
        rec128 = np.ascontiguousarray(rec128.reshape(128, N_NTF)).astype(NPBF)

        in_maps.append({
            "attr2": attr_blk,
            "xt0": np.ascontiguousarray(xt[0:128]),
            "xt1": np.ascontiguousarray(xt[128:256]),
            "rec": rec128,
            "w2": w2,
            "wd": wd,
            "wf": wf,
            "wu": wu,
            "fold": fold,
            "eyeI": eyeI,
            "biases": biases,
        })
    return in_maps, store_cols


def run(in_maps, trace=False, **kw):
    nc = _build_graph()
    return run_bass_kernel_spmd(nc, in_maps, core_ids=list(range(NCORES)),
                                trace=trace, **kw)


def unshard(results, store_cols):
    out = np.empty((N_NODES, IN_CH), dtype=np.float32)
    for c in range(NCORES):
        o = np.asarray(results[c]["out"], dtype=np.float32)  # [256, N_STORE]
        out[c * NC_NODES:(c + 1) * NC_NODES] = o[:, store_cols[c]].T
    return out


def kernel(**inputs):
    in_maps, store_cols = prepare(**inputs)
    res = run(in_maps, trace=False)
    return unshard(res.results, store_cols)
